# revision 38
# baseline (speedup 1.0000x reference)
"""MoE (top-2 of 8 experts, SwiGLU) Trainium2 kernel.

Strategy (quarter-sliced expert parallelism over 8 NeuronCores):
  * SwiGLU is elementwise in the intermediate dim I, so an expert's FFN
    splits cleanly into 4 independent I-slices of 512: each slice computes
    h_q = silu(x@Wg[:,q]) * (x@Wu[:,q]); y = sum_q h_q @ Wd[q,:].  Each core
    hosts 4 quarter-experts (12 MB bf16, same as one full expert).  Experts
    are paired by sorted token count ((1st,2nd), (3rd,4th), ...) and each
    pair's slices spread over one 4-core group, so per-core token capacity
    drops from max_e count_e (2182) to c1+c3+c5+c7 quarters (8278/4 ~ 2070
    token-equivalents) - near-perfect load balance.
  * Host: router GEMM + top-2 + sigmoid gates in numpy (matches the jax fp32
    reference bit-compatibly); gathers per-expert tokens into slot-packed
    bf16 buffers; applies gates and sums the 4 quarter partials on the host
    during the final scatter-add (all fp32).
  * Device (SPMD, per core): fused single pass per slot, all matmuls bf16
    (full PE rate; bf16 self-loading matmuls have no per-instruction weight
    penalty, unlike fp32r's ~13.5ns).  Weights stay SBUF-resident; x streams
    per superchunk; h lives in SBUF (no DRAM spill); y written as bf16.
    Dummy N=128 matmuls on a zeroed tile fill the unavoidable ~10us DMA head
    so the PE's HAM clock gate stays warm.
"""

import os
import numpy as np
import ml_dtypes

T, H, I, E, TOPK = 8192, 1024, 2048, 8, 2
NCORES = 8
PB = 128
NSLOT = 4
QI = I // NSLOT          # 512: I-columns per slot
QIB = QI // PB           # 4 i-blocks per slot

_compiled = {}
last_results = None  # BassKernelResults of the most recent run (for test harness)

BF16 = ml_dtypes.bfloat16


def _segs(width):
    segs = []
    t0 = 0
    while t0 < width:
        w = min(512, width - t0)
        segs.append((t0, w))
        t0 += w
    return segs


def _build(widths):
    import concourse.bacc as bacc
    import concourse.mybir as mybir
    import concourse.tile as tile

    fp32 = mybir.dt.float32
    bf16 = mybir.dt.bfloat16
    AF = mybir.ActivationFunctionType

    KB = H // PB   # 8 contraction blocks over H
    HB = H // PB   # 8 output blocks over H
    SW = sum(widths)
    offs = [sum(widths[:j]) for j in range(NSLOT)]

    nc = bacc.Bacc("TRN2", target_bir_lowering=False, debug=False,
                   num_devices=NCORES)
    xT = nc.dram_tensor("xT", [H, SW], bf16, kind="ExternalInput").ap()
    Wg = nc.dram_tensor("Wg", [H, I], bf16, kind="ExternalInput").ap()
    Wu = nc.dram_tensor("Wu", [H, I], bf16, kind="ExternalInput").ap()
    Wd = nc.dram_tensor("Wd", [I, H], bf16, kind="ExternalInput").ap()
    yT = nc.dram_tensor("yT", [H, SW], bf16, kind="ExternalOutput").ap()
    dbg = nc.dram_tensor("dbg", [PB, 512], fp32, kind="ExternalOutput").ap()

    slot_segs = [_segs(w) for w in widths]
    slot_scs = [[s[i:i + 2] for i in range(0, len(s), 2)] for s in slot_segs]

    with tile.TileContext(nc) as tc:
        with tc.tile_pool(name="wp", bufs=1) as wp, \
             tc.tile_pool(name="xp", bufs=1) as xp, \
             tc.tile_pool(name="hp", bufs=1) as hp, \
             tc.tile_pool(name="evp", bufs=4) as evp, \
             tc.tile_pool(name="yp", bufs=4) as yp, \
             tc.tile_pool(name="psA", bufs=1, space="PSUM") as psA, \
             tc.tile_pool(name="psB", bufs=2, space="PSUM") as psB:

            # --- PE warmup / gap fillers: one long accumulation group of
            # dummy matmuls on a zeroed tile, batches interleaved with the
            # DMA-gated head so the HAM clock gate never sees an idle window.
            wz = wp.tile([PB, 512], bf16, name="wz")
            nc.gpsimd.memset(wz[:], 0)
            # fillers live in one psB "py0" slot (bufs=2): phase 2 of the
            # first superchunk starts after the filler group closes.
            pw = psB.tile([PB, 512], fp32, tag="py0", name="pw")
            fill_state = {"started": False, "closed": False}

            def filler(n, last=False):
                if fill_state["closed"]:
                    return
                for j in range(n):
                    nc.tensor.matmul(pw[:, 0:PB], wz[:, 0:PB], wz[:, 0:PB],
                                     start=(not fill_state["started"]),
                                     stop=last and (j == n - 1))
                    fill_state["started"] = True
                if last:
                    fill_state["closed"] = True
                    yw = wp.tile([PB, 512], fp32, name="yw")
                    nc.vector.tensor_copy(yw[:, 0:PB], pw[:, 0:PB])
                    nc.gpsimd.dma_start(out=dbg[:, 0:PB], in_=yw[:, 0:PB])

            filler(36)

            # --- DMA schedule.  Each dma_start costs the issuing sequencer
            # ~600ns regardless of size, so transfers are column-
            # consolidated: weights q1-3 as single [128,1536] rows, x as one
            # [128, w] tile per (k, superchunk).  x tiles use a depth-3 tag
            # pipeline (mod-3 tags, bufs=1): an x issue unblocks 3
            # superchunks ahead of need, so issue bursts at buffer-release
            # points never starve anything downstream in the queue.
            xsc = {}   # (k, global_sc) -> tile
            wg0, wu0, wgR, wuR = {}, {}, {}, {}
            wd_s = {}

            # global superchunk list: (slot, sci, col0, width)
            gsc = []
            for sj in range(NSLOT):
                for sci, sc in enumerate(slot_scs[sj]):
                    col0 = offs[sj] + sc[0][0]
                    wtot = sum(w for _, w in sc)
                    gsc.append((sj, sci, col0, wtot))

            def dma_x_sc(g):
                sj, sci, col0, wtot = gsc[g]
                for k in range(KB):
                    xt = xp.tile([PB, wtot], bf16, tag=f"x{k}_{g % 3}",
                                 name=f"x{k}_g{g}")
                    nc.sync.dma_start(out=xt[:],
                                      in_=xT[k * PB:(k + 1) * PB,
                                             col0:col0 + wtot])
                    xsc[(k, g)] = xt

            def dma_wd(q):
                for ib in range(q * QIB, (q + 1) * QIB):
                    wdt = wp.tile([PB, H], bf16, name=f"wd{ib}")
                    nc.sync.dma_start(out=wdt[:],
                                      in_=Wd[ib * PB:(ib + 1) * PB, :])
                    wd_s[ib] = wdt

            # Head: x (sc0, both segs in one tile per k) interleaved with
            # Wg q0, then Wu q0.
            _, _, col00, w00 = gsc[0]
            for k in range(KB):
                xt = xp.tile([PB, w00], bf16, tag=f"x{k}_0", name=f"x{k}_g0")
                nc.sync.dma_start(out=xt[:],
                                  in_=xT[k * PB:(k + 1) * PB,
                                         col00:col00 + w00])
                xsc[(k, 0)] = xt
                wgt = wp.tile([PB, QI], bf16, name=f"wg{k}_0")
                nc.sync.dma_start(out=wgt[:], in_=Wg[k * PB:(k + 1) * PB,
                                                     0:QI])
                wg0[k] = wgt
            # Wu q0 issues on the scalar HWDGE queue: ~600ns sequencer cost
            # per descriptor runs in parallel with sync's x+Wg stream, and
            # the scalar engine's first silu comes well after.
            for k in range(KB):
                wut = wp.tile([PB, QI], bf16, name=f"wu{k}_0")
                nc.scalar.dma_start(out=wut[:], in_=Wu[k * PB:(k + 1) * PB,
                                                       0:QI])
                wu0[k] = wut
            dma_wd(0)
            dma_x_sc(1)
            for k in range(KB):
                wgt = wp.tile([PB, I - QI], bf16, name=f"wgR{k}")
                nc.sync.dma_start(out=wgt[:],
                                  in_=Wg[k * PB:(k + 1) * PB, QI:I])
                wgR[k] = wgt
            for k in range(KB):
                wut = wp.tile([PB, I - QI], bf16, name=f"wuR{k}")
                nc.sync.dma_start(out=wut[:],
                                  in_=Wu[k * PB:(k + 1) * PB, QI:I])
                wuR[k] = wut
            dma_x_sc(2)
            dma_wd(1)
            dma_wd(2)
            dma_wd(3)
            for g in range(3, len(gsc)):
                dma_x_sc(g)

            def wg_slice(k, q, lo, hi):
                if q == 0:
                    return wg0[k][:, lo:hi]
                return wgR[k][:, (q - 1) * QI + lo:(q - 1) * QI + hi]

            def wu_slice(k, q, lo, hi):
                if q == 0:
                    return wu0[k][:, lo:hi]
                return wuR[k][:, (q - 1) * QI + lo:(q - 1) * QI + hi]

            # --- fused compute, one slot (quarter-expert) at a time
            h_tiles = {}   # (g, ib, sl) -> tile
            sx_tiles = {}

            def emit_pg(g, ib, sl, sl_tag):
                sj, sci, col0, wtot = gsc[g]
                sc = slot_scs[sj][sci]
                t0, w = sc[sl]
                lo = t0 - sc[0][0]
                pg = psA.tile([PB, w], fp32, tag=f"pg{sl_tag % 2}",
                              name=f"pg{sl_tag}")
                for k in range(KB):
                    nc.tensor.matmul(
                        pg[:],
                        wg_slice(k, sj, ib * PB, (ib + 1) * PB),
                        xsc[(k, g)][:, lo:lo + w],
                        start=(k == 0), stop=(k == KB - 1))
                sx = evp.tile([PB, w], fp32, tag=f"sx{sl_tag % 2}",
                              name=f"sx{sl_tag}")
                nc.scalar.activation(sx[:], pg[:], AF.Silu)
                sx_tiles[(g, ib, sl)] = sx

            def emit_pu(g, ib, sl, sl_tag):
                sj, sci, col0, wtot = gsc[g]
                sc = slot_scs[sj][sci]
                t0, w = sc[sl]
                lo = t0 - sc[0][0]
                pu = psA.tile([PB, w], fp32, tag=f"pu{sl_tag % 2}",
                              name=f"pu{sl_tag}")
                for k in range(KB):
                    nc.tensor.matmul(
                        pu[:],
                        wu_slice(k, sj, ib * PB, (ib + 1) * PB),
                        xsc[(k, g)][:, lo:lo + w],
                        start=(k == 0), stop=(k == KB - 1))
                hh = hp.tile([PB, w], bf16, tag=f"h{ib}_{sl_tag}",
                             name=f"h{ib}_{sl_tag}")
                nc.vector.tensor_mul(hh[:], sx_tiles.pop((g, ib, sl))[:],
                                     pu[:])
                h_tiles[(g, ib, sl)] = hh

            def emit_p1(g, tiny=False):
                sj, sci, _, _ = gsc[g]
                sc = slot_scs[sj][sci]
                if g == 0:
                    # Head: pg-groups first (need only x g0 + Wg q0 =
                    # first 3 MB of DMA); filler batches bridge the
                    # DMA-gated stretches so HAM stays warm.
                    for sl in range(len(sc)):
                        for ib in range(QIB):
                            emit_pg(g, ib, sl, sl)
                        filler(16)
                        for ib in range(QIB):
                            emit_pu(g, ib, sl, sl)
                        if sl == 0:
                            filler(16)
                    filler(8, last=True)
                else:
                    for ib in range(QIB):
                        for sl in range(len(sc)):
                            sl_tag = 2 if tiny else sl
                            emit_pg(g, ib, sl, sl_tag)
                            emit_pu(g, ib, sl, sl_tag)

            def emit_p2(g):
                # Phase 2: y = h @ Wd (gates applied on host).  Both
                # segments' evictions share one yt tile and one DMA.
                sj, sci, col0, wtot = gsc[g]
                sc = slot_scs[sj][sci]
                for hb in range(HB):
                    yt = yp.tile([PB, wtot], bf16, tag="yt", name="yt")
                    for sl, (t0, w) in enumerate(sc):
                        lo = t0 - sc[0][0]
                        py = psB.tile([PB, w], fp32, tag=f"py{sl}",
                                      name=f"py{sl}")
                        for il in range(QIB):
                            nc.tensor.matmul(
                                py[:],
                                wd_s[sj * QIB + il][:,
                                                    hb * PB:(hb + 1) * PB],
                                h_tiles[(g, il, sl)][:],
                                start=(il == 0), stop=(il == QIB - 1))
                        nc.vector.tensor_copy(yt[:, lo:lo + w], py[:])
                    nc.gpsimd.dma_start(
                        out=yT[hb * PB:(hb + 1) * PB, col0:col0 + wtot],
                        in_=yt[:])

            # Plain superchunk order.  (Hoisting a trailing single-seg
            # superchunk's phase 1 ahead of the previous phase 2 was tried
            # to hide its silu/h chain, but perturbed the scheduler and
            # measured ~2-3us worse overall.)
            for g in range(len(gsc)):
                emit_p1(g)
                emit_p2(g)

    nc.compile()
    return nc


def _route(x, Wr, br):
    """Replicate the reference's fp32 router bit-compatibly on host."""
    logits = x @ Wr + br                       # fp32 GEMM
    order = np.argsort(-logits, axis=1, kind="stable")  # ties -> lowest index
    topk_idx = order[:, :TOPK]
    topk_vals = np.take_along_axis(logits, topk_idx, axis=1)
    g = 1.0 / (1.0 + np.exp(-topk_vals.astype(np.float32)))
    g = g / (np.sum(g, axis=-1, keepdims=True) + 1e-10)
    return topk_idx, g.astype(np.float32)


def kernel(x, Wr, br, Wg, Wu, Wd):
    global last_results
    from concourse.bass_utils import run_bass_kernel_spmd

    x = np.asarray(x, dtype=np.float32)
    Wr = np.asarray(Wr, dtype=np.float32)
    br = np.asarray(br, dtype=np.float32)
    Wg = np.asarray(Wg, dtype=np.float32)
    Wu = np.asarray(Wu, dtype=np.float32)
    Wd = np.asarray(Wd, dtype=np.float32)

    topk_idx, g = _route(x, Wr, br)

    idx_lists = []
    gate_lists = []
    for e in range(E):
        mask = topk_idx == e                    # [T, K]
        tok = np.nonzero(mask.any(axis=1))[0]
        gsel = np.where(mask[tok, 0], g[tok, 0], g[tok, 1]).astype(np.float32)
        idx_lists.append(tok.astype(np.int64))
        gate_lists.append(gsel)

    counts = np.array([len(ix) for ix in idx_lists])
    ranked = np.argsort(-counts, kind="stable")
    # slot j: experts ranked[2j] (cores 0-3) / ranked[2j+1] (cores 4-7);
    # core c hosts quarter (c % 4) of each of its slot experts.
    widths = tuple(max(512, int(counts[ranked[2 * j]])) for j in range(NSLOT))
    offs = [sum(widths[:j]) for j in range(NSLOT)]
    SW = sum(widths)

    key = widths
    if key not in _compiled:
        _compiled[key] = _build(widths)
    nc = _compiled[key]

    xTb = np.ascontiguousarray(x.T).astype(BF16)   # [H, T] bf16
    Wg16 = [Wg[e].astype(BF16) for e in range(E)]
    Wu16 = [Wu[e].astype(BF16) for e in range(E)]
    Wd16 = [Wd[e].astype(BF16) for e in range(E)]

    in_maps = []
    slot_expert = np.zeros((NCORES, NSLOT), dtype=int)
    for c in range(NCORES):
        qt = c % 4
        xTe = np.zeros((H, SW), dtype=BF16)
        Wg_in = np.zeros((H, I), dtype=BF16)
        Wu_in = np.zeros((H, I), dtype=BF16)
        Wd_in = np.zeros((I, H), dtype=BF16)
        for j in range(NSLOT):
            e = int(ranked[2 * j + (0 if c < 4 else 1)])
            slot_expert[c, j] = e
            n = counts[e]
            xTe[:, offs[j]:offs[j] + n] = xTb[:, idx_lists[e]]
            Wg_in[:, j * QI:(j + 1) * QI] = Wg16[e][:, qt * QI:(qt + 1) * QI]
            Wu_in[:, j * QI:(j + 1) * QI] = Wu16[e][:, qt * QI:(qt + 1) * QI]
            Wd_in[j * QI:(j + 1) * QI, :] = Wd16[e][qt * QI:(qt + 1) * QI, :]
        in_maps.append({"xT": xTe, "Wg": Wg_in, "Wu": Wu_in, "Wd": Wd_in})

    trace = bool(int(os.environ.get("MOE_TRACE", "0")))
    trace_cores = (list(range(NCORES))
                   if os.environ.get("MOE_TRACE_ALL") else None)
    last_results = run_bass_kernel_spmd(
        nc, in_maps, core_ids=list(range(NCORES)), trace=trace,
        trace_cores=trace_cores)

    out = np.zeros((T, H), dtype=np.float32)
    for j in range(NSLOT):
        for half, cores in ((0, range(0, 4)), (1, range(4, 8))):
            e = int(ranked[2 * j + half])
            n = counts[e]
            acc = np.zeros((n, H), dtype=np.float32)
            for c in cores:
                yTe = last_results.results[c]["yT"]
                acc += yTe[:, offs[j]:offs[j] + n].T.astype(np.float32)
            out[idx_lists[e]] += acc * gate_lists[e][:, None]
    return out


# revision 39
# speedup vs baseline: 1.0075x; 1.0075x over previous
"""MoE (top-2 of 8 experts, SwiGLU) Trainium2 kernel.

Strategy (quarter-sliced expert parallelism over 8 NeuronCores):
  * SwiGLU is elementwise in the intermediate dim I, so an expert's FFN
    splits cleanly into 4 independent I-slices of 512: each slice computes
    h_q = silu(x@Wg[:,q]) * (x@Wu[:,q]); y = sum_q h_q @ Wd[q,:].  Each core
    hosts 4 quarter-experts (12 MB bf16, same as one full expert).  Experts
    are paired by sorted token count ((1st,2nd), (3rd,4th), ...) and each
    pair's slices spread over one 4-core group, so per-core token capacity
    drops from max_e count_e (2182) to c1+c3+c5+c7 quarters (8278/4 ~ 2070
    token-equivalents) - near-perfect load balance.
  * Host: router GEMM + top-2 + sigmoid gates in numpy (matches the jax fp32
    reference bit-compatibly); gathers per-expert tokens into slot-packed
    bf16 buffers; applies gates and sums the 4 quarter partials on the host
    during the final scatter-add (all fp32).
  * Device (SPMD, per core): fused single pass per slot, all matmuls bf16
    (full PE rate; bf16 self-loading matmuls have no per-instruction weight
    penalty, unlike fp32r's ~13.5ns).  Weights stay SBUF-resident; x streams
    per superchunk; h lives in SBUF (no DRAM spill); y written as bf16.
    Dummy N=128 matmuls on a zeroed tile fill the unavoidable ~10us DMA head
    so the PE's HAM clock gate stays warm.
"""

import os
import numpy as np
import ml_dtypes

T, H, I, E, TOPK = 8192, 1024, 2048, 8, 2
NCORES = 8
PB = 128
NSLOT = 4
QI = I // NSLOT          # 512: I-columns per slot
QIB = QI // PB           # 4 i-blocks per slot

_compiled = {}
last_results = None  # BassKernelResults of the most recent run (for test harness)

BF16 = ml_dtypes.bfloat16


def _segs(width):
    segs = []
    t0 = 0
    while t0 < width:
        w = min(512, width - t0)
        segs.append((t0, w))
        t0 += w
    return segs


def _build(widths):
    import concourse.bacc as bacc
    import concourse.mybir as mybir
    import concourse.tile as tile

    fp32 = mybir.dt.float32
    bf16 = mybir.dt.bfloat16
    AF = mybir.ActivationFunctionType

    KB = H // PB   # 8 contraction blocks over H
    HB = H // PB   # 8 output blocks over H
    SW = sum(widths)
    offs = [sum(widths[:j]) for j in range(NSLOT)]

    nc = bacc.Bacc("TRN2", target_bir_lowering=False, debug=False,
                   num_devices=NCORES)
    xT = nc.dram_tensor("xT", [H, SW], bf16, kind="ExternalInput").ap()
    Wg = nc.dram_tensor("Wg", [H, I], bf16, kind="ExternalInput").ap()
    Wu = nc.dram_tensor("Wu", [H, I], bf16, kind="ExternalInput").ap()
    Wd = nc.dram_tensor("Wd", [I, H], bf16, kind="ExternalInput").ap()
    yT = nc.dram_tensor("yT", [H, SW], bf16, kind="ExternalOutput").ap()
    dbg = nc.dram_tensor("dbg", [PB, 512], fp32, kind="ExternalOutput").ap()

    slot_segs = [_segs(w) for w in widths]
    slot_scs = [[s[i:i + 2] for i in range(0, len(s), 2)] for s in slot_segs]

    with tile.TileContext(nc) as tc:
        with tc.tile_pool(name="wp", bufs=1) as wp, \
             tc.tile_pool(name="xp", bufs=1) as xp, \
             tc.tile_pool(name="hp", bufs=1) as hp, \
             tc.tile_pool(name="evp", bufs=4) as evp, \
             tc.tile_pool(name="yp", bufs=4) as yp, \
             tc.tile_pool(name="psA", bufs=1, space="PSUM") as psA, \
             tc.tile_pool(name="psB", bufs=2, space="PSUM") as psB:

            # --- PE warmup / gap fillers: one long accumulation group of
            # dummy matmuls on a zeroed tile, batches interleaved with the
            # DMA-gated head so the HAM clock gate never sees an idle window.
            wz = wp.tile([PB, 512], bf16, name="wz")
            nc.gpsimd.memset(wz[:], 0)
            # fillers live in one psB "py0" slot (bufs=2): phase 2 of the
            # first superchunk starts after the filler group closes.
            pw = psB.tile([PB, 512], fp32, tag="py0", name="pw")
            fill_state = {"started": False, "closed": False}

            def filler(n, last=False):
                if fill_state["closed"]:
                    return
                for j in range(n):
                    nc.tensor.matmul(pw[:, 0:PB], wz[:, 0:PB], wz[:, 0:PB],
                                     start=(not fill_state["started"]),
                                     stop=last and (j == n - 1))
                    fill_state["started"] = True
                if last:
                    fill_state["closed"] = True
                    yw = wp.tile([PB, 512], fp32, name="yw")
                    nc.vector.tensor_copy(yw[:, 0:PB], pw[:, 0:PB])
                    nc.gpsimd.dma_start(out=dbg[:, 0:PB], in_=yw[:, 0:PB])

            filler(36)

            # --- DMA schedule.  Each dma_start costs the issuing sequencer
            # ~600ns regardless of size, so transfers are column-
            # consolidated: weights q1-3 as single [128,1536] rows, x as one
            # [128, w] tile per (k, superchunk).  x tiles use a depth-3 tag
            # pipeline (mod-3 tags, bufs=1): an x issue unblocks 3
            # superchunks ahead of need, so issue bursts at buffer-release
            # points never starve anything downstream in the queue.
            xsc = {}   # (k, global_sc) -> tile
            wg0, wu0, wgR, wuR = {}, {}, {}, {}
            wd_s = {}

            # global superchunk list: (slot, sci, col0, width)
            gsc = []
            for sj in range(NSLOT):
                for sci, sc in enumerate(slot_scs[sj]):
                    col0 = offs[sj] + sc[0][0]
                    wtot = sum(w for _, w in sc)
                    gsc.append((sj, sci, col0, wtot))

            def dma_x_sc(g):
                sj, sci, col0, wtot = gsc[g]
                for k in range(KB):
                    xt = xp.tile([PB, wtot], bf16, tag=f"x{k}_{g % 3}",
                                 name=f"x{k}_g{g}")
                    nc.sync.dma_start(out=xt[:],
                                      in_=xT[k * PB:(k + 1) * PB,
                                             col0:col0 + wtot])
                    xsc[(k, g)] = xt

            def dma_wd(q):
                for ib in range(q * QIB, (q + 1) * QIB):
                    wdt = wp.tile([PB, H], bf16, name=f"wd{ib}")
                    nc.sync.dma_start(out=wdt[:],
                                      in_=Wd[ib * PB:(ib + 1) * PB, :])
                    wd_s[ib] = wdt

            # Head: x (sc0, both segs in one tile per k) interleaved with
            # Wg q0, then Wu q0.
            _, _, col00, w00 = gsc[0]
            for k in range(KB):
                xt = xp.tile([PB, w00], bf16, tag=f"x{k}_0", name=f"x{k}_g0")
                nc.sync.dma_start(out=xt[:],
                                  in_=xT[k * PB:(k + 1) * PB,
                                         col00:col00 + w00])
                xsc[(k, 0)] = xt
                wgt = wp.tile([PB, QI], bf16, name=f"wg{k}_0")
                nc.sync.dma_start(out=wgt[:], in_=Wg[k * PB:(k + 1) * PB,
                                                     0:QI])
                wg0[k] = wgt
            for k in range(KB):
                wut = wp.tile([PB, QI], bf16, name=f"wu{k}_0")
                nc.sync.dma_start(out=wut[:], in_=Wu[k * PB:(k + 1) * PB,
                                                     0:QI])
                wu0[k] = wut
            dma_wd(0)
            dma_x_sc(1)
            for k in range(KB):
                wgt = wp.tile([PB, I - QI], bf16, name=f"wgR{k}")
                nc.sync.dma_start(out=wgt[:],
                                  in_=Wg[k * PB:(k + 1) * PB, QI:I])
                wgR[k] = wgt
            for k in range(KB):
                wut = wp.tile([PB, I - QI], bf16, name=f"wuR{k}")
                nc.sync.dma_start(out=wut[:],
                                  in_=Wu[k * PB:(k + 1) * PB, QI:I])
                wuR[k] = wut
            dma_x_sc(2)
            dma_wd(1)
            dma_wd(2)
            dma_wd(3)
            for g in range(3, len(gsc)):
                dma_x_sc(g)

            def wg_slice(k, q, lo, hi):
                if q == 0:
                    return wg0[k][:, lo:hi]
                return wgR[k][:, (q - 1) * QI + lo:(q - 1) * QI + hi]

            def wu_slice(k, q, lo, hi):
                if q == 0:
                    return wu0[k][:, lo:hi]
                return wuR[k][:, (q - 1) * QI + lo:(q - 1) * QI + hi]

            # --- fused compute, one slot (quarter-expert) at a time
            h_tiles = {}   # (g, ib, sl) -> tile
            sx_tiles = {}

            def emit_pg(g, ib, sl, sl_tag):
                sj, sci, col0, wtot = gsc[g]
                sc = slot_scs[sj][sci]
                t0, w = sc[sl]
                lo = t0 - sc[0][0]
                pg = psA.tile([PB, w], fp32, tag=f"pg{sl_tag % 2}",
                              name=f"pg{sl_tag}")
                for k in range(KB):
                    nc.tensor.matmul(
                        pg[:],
                        wg_slice(k, sj, ib * PB, (ib + 1) * PB),
                        xsc[(k, g)][:, lo:lo + w],
                        start=(k == 0), stop=(k == KB - 1))
                sx = evp.tile([PB, w], fp32, tag=f"sx{sl_tag % 2}",
                              name=f"sx{sl_tag}")
                nc.scalar.activation(sx[:], pg[:], AF.Silu)
                sx_tiles[(g, ib, sl)] = sx

            def emit_pu(g, ib, sl, sl_tag):
                sj, sci, col0, wtot = gsc[g]
                sc = slot_scs[sj][sci]
                t0, w = sc[sl]
                lo = t0 - sc[0][0]
                pu = psA.tile([PB, w], fp32, tag=f"pu{sl_tag % 2}",
                              name=f"pu{sl_tag}")
                for k in range(KB):
                    nc.tensor.matmul(
                        pu[:],
                        wu_slice(k, sj, ib * PB, (ib + 1) * PB),
                        xsc[(k, g)][:, lo:lo + w],
                        start=(k == 0), stop=(k == KB - 1))
                hh = hp.tile([PB, w], bf16, tag=f"h{ib}_{sl_tag}",
                             name=f"h{ib}_{sl_tag}")
                nc.vector.tensor_mul(hh[:], sx_tiles.pop((g, ib, sl))[:],
                                     pu[:])
                h_tiles[(g, ib, sl)] = hh

            def emit_p1(g, tiny=False):
                sj, sci, _, _ = gsc[g]
                sc = slot_scs[sj][sci]
                if g == 0:
                    # Head: pg-groups first (need only x g0 + Wg q0 =
                    # first 3 MB of DMA); filler batches bridge the
                    # DMA-gated stretches so HAM stays warm.
                    for sl in range(len(sc)):
                        for ib in range(QIB):
                            emit_pg(g, ib, sl, sl)
                        filler(16)
                        for ib in range(QIB):
                            emit_pu(g, ib, sl, sl)
                        if sl == 0:
                            filler(16)
                    filler(8, last=True)
                else:
                    for ib in range(QIB):
                        for sl in range(len(sc)):
                            sl_tag = 2 if tiny else sl
                            emit_pg(g, ib, sl, sl_tag)
                            emit_pu(g, ib, sl, sl_tag)

            def emit_p2(g):
                # Phase 2: y = h @ Wd (gates applied on host).  Both
                # segments' evictions share one yt tile and one DMA.
                sj, sci, col0, wtot = gsc[g]
                sc = slot_scs[sj][sci]
                for hb in range(HB):
                    yt = yp.tile([PB, wtot], bf16, tag="yt", name="yt")
                    for sl, (t0, w) in enumerate(sc):
                        lo = t0 - sc[0][0]
                        py = psB.tile([PB, w], fp32, tag=f"py{sl}",
                                      name=f"py{sl}")
                        for il in range(QIB):
                            nc.tensor.matmul(
                                py[:],
                                wd_s[sj * QIB + il][:,
                                                    hb * PB:(hb + 1) * PB],
                                h_tiles[(g, il, sl)][:],
                                start=(il == 0), stop=(il == QIB - 1))
                        nc.vector.tensor_copy(yt[:, lo:lo + w], py[:])
                    nc.gpsimd.dma_start(
                        out=yT[hb * PB:(hb + 1) * PB, col0:col0 + wtot],
                        in_=yt[:])

            # Plain superchunk order.  (Hoisting a trailing single-seg
            # superchunk's phase 1 ahead of the previous phase 2 was tried
            # to hide its silu/h chain, but perturbed the scheduler and
            # measured ~2-3us worse overall.)
            for g in range(len(gsc)):
                emit_p1(g)
                emit_p2(g)

    nc.compile()
    return nc


def _route(x, Wr, br):
    """Replicate the reference's fp32 router bit-compatibly on host."""
    logits = x @ Wr + br                       # fp32 GEMM
    order = np.argsort(-logits, axis=1, kind="stable")  # ties -> lowest index
    topk_idx = order[:, :TOPK]
    topk_vals = np.take_along_axis(logits, topk_idx, axis=1)
    g = 1.0 / (1.0 + np.exp(-topk_vals.astype(np.float32)))
    g = g / (np.sum(g, axis=-1, keepdims=True) + 1e-10)
    return topk_idx, g.astype(np.float32)


def kernel(x, Wr, br, Wg, Wu, Wd):
    global last_results
    from concourse.bass_utils import run_bass_kernel_spmd

    x = np.asarray(x, dtype=np.float32)
    Wr = np.asarray(Wr, dtype=np.float32)
    br = np.asarray(br, dtype=np.float32)
    Wg = np.asarray(Wg, dtype=np.float32)
    Wu = np.asarray(Wu, dtype=np.float32)
    Wd = np.asarray(Wd, dtype=np.float32)

    topk_idx, g = _route(x, Wr, br)

    idx_lists = []
    gate_lists = []
    for e in range(E):
        mask = topk_idx == e                    # [T, K]
        tok = np.nonzero(mask.any(axis=1))[0]
        gsel = np.where(mask[tok, 0], g[tok, 0], g[tok, 1]).astype(np.float32)
        idx_lists.append(tok.astype(np.int64))
        gate_lists.append(gsel)

    counts = np.array([len(ix) for ix in idx_lists])
    ranked = np.argsort(-counts, kind="stable")
    # slot j: experts ranked[2j] (cores 0-3) / ranked[2j+1] (cores 4-7);
    # core c hosts quarter (c % 4) of each of its slot experts.
    widths = tuple(max(512, int(counts[ranked[2 * j]])) for j in range(NSLOT))
    offs = [sum(widths[:j]) for j in range(NSLOT)]
    SW = sum(widths)

    key = widths
    if key not in _compiled:
        _compiled[key] = _build(widths)
    nc = _compiled[key]

    xTb = np.ascontiguousarray(x.T).astype(BF16)   # [H, T] bf16
    Wg16 = [Wg[e].astype(BF16) for e in range(E)]
    Wu16 = [Wu[e].astype(BF16) for e in range(E)]
    Wd16 = [Wd[e].astype(BF16) for e in range(E)]

    in_maps = []
    slot_expert = np.zeros((NCORES, NSLOT), dtype=int)
    for c in range(NCORES):
        qt = c % 4
        xTe = np.zeros((H, SW), dtype=BF16)
        Wg_in = np.zeros((H, I), dtype=BF16)
        Wu_in = np.zeros((H, I), dtype=BF16)
        Wd_in = np.zeros((I, H), dtype=BF16)
        for j in range(NSLOT):
            e = int(ranked[2 * j + (0 if c < 4 else 1)])
            slot_expert[c, j] = e
            n = counts[e]
            xTe[:, offs[j]:offs[j] + n] = xTb[:, idx_lists[e]]
            Wg_in[:, j * QI:(j + 1) * QI] = Wg16[e][:, qt * QI:(qt + 1) * QI]
            Wu_in[:, j * QI:(j + 1) * QI] = Wu16[e][:, qt * QI:(qt + 1) * QI]
            Wd_in[j * QI:(j + 1) * QI, :] = Wd16[e][qt * QI:(qt + 1) * QI, :]
        in_maps.append({"xT": xTe, "Wg": Wg_in, "Wu": Wu_in, "Wd": Wd_in})

    trace = bool(int(os.environ.get("MOE_TRACE", "0")))
    trace_cores = (list(range(NCORES))
                   if os.environ.get("MOE_TRACE_ALL") else None)
    last_results = run_bass_kernel_spmd(
        nc, in_maps, core_ids=list(range(NCORES)), trace=trace,
        trace_cores=trace_cores)

    out = np.zeros((T, H), dtype=np.float32)
    for j in range(NSLOT):
        for half, cores in ((0, range(0, 4)), (1, range(4, 8))):
            e = int(ranked[2 * j + half])
            n = counts[e]
            acc = np.zeros((n, H), dtype=np.float32)
            for c in cores:
                yTe = last_results.results[c]["yT"]
                acc += yTe[:, offs[j]:offs[j] + n].T.astype(np.float32)
            out[idx_lists[e]] += acc * gate_lists[e][:, None]
    return out


# revision 40
# speedup vs baseline: 1.0294x; 1.0217x over previous
"""MoE (top-2 of 8 experts, SwiGLU) Trainium2 kernel.

Strategy (quarter-sliced expert parallelism over 8 NeuronCores):
  * SwiGLU is elementwise in the intermediate dim I, so an expert's FFN
    splits cleanly into 4 independent I-slices of 512: each slice computes
    h_q = silu(x@Wg[:,q]) * (x@Wu[:,q]); y = sum_q h_q @ Wd[q,:].  Each core
    hosts 4 quarter-experts (12 MB bf16, same as one full expert).  Experts
    are paired by sorted token count ((1st,2nd), (3rd,4th), ...) and each
    pair's slices spread over one 4-core group, so per-core token capacity
    drops from max_e count_e (2182) to c1+c3+c5+c7 quarters (8278/4 ~ 2070
    token-equivalents) - near-perfect load balance.
  * Host: router GEMM + top-2 + sigmoid gates in numpy (matches the jax fp32
    reference bit-compatibly); gathers per-expert tokens into slot-packed
    bf16 buffers; applies gates and sums the 4 quarter partials on the host
    during the final scatter-add (all fp32).
  * Device (SPMD, per core): fused single pass per slot, all matmuls bf16
    (full PE rate; bf16 self-loading matmuls have no per-instruction weight
    penalty, unlike fp32r's ~13.5ns).  Weights stay SBUF-resident; x streams
    per superchunk; h lives in SBUF (no DRAM spill); y written as bf16.
    Dummy N=128 matmuls on a zeroed tile fill the unavoidable ~10us DMA head
    so the PE's HAM clock gate stays warm.
"""

import os
import numpy as np
import ml_dtypes

T, H, I, E, TOPK = 8192, 1024, 2048, 8, 2
NCORES = 8
PB = 128
NSLOT = 4
QI = I // NSLOT          # 512: I-columns per slot
QIB = QI // PB           # 4 i-blocks per slot

_compiled = {}
last_results = None  # BassKernelResults of the most recent run (for test harness)

BF16 = ml_dtypes.bfloat16


def _segs(width):
    segs = []
    t0 = 0
    while t0 < width:
        w = min(512, width - t0)
        segs.append((t0, w))
        t0 += w
    return segs


def _build(widths):
    import concourse.bacc as bacc
    import concourse.mybir as mybir
    import concourse.tile as tile

    fp32 = mybir.dt.float32
    bf16 = mybir.dt.bfloat16
    AF = mybir.ActivationFunctionType

    KB = H // PB   # 8 contraction blocks over H
    HB = H // PB   # 8 output blocks over H
    SW = sum(widths)
    offs = [sum(widths[:j]) for j in range(NSLOT)]

    nc = bacc.Bacc("TRN2", target_bir_lowering=False, debug=False,
                   num_devices=NCORES)
    xT = nc.dram_tensor("xT", [H, SW], bf16, kind="ExternalInput").ap()
    Wg = nc.dram_tensor("Wg", [H, I], bf16, kind="ExternalInput").ap()
    Wu = nc.dram_tensor("Wu", [H, I], bf16, kind="ExternalInput").ap()
    Wd = nc.dram_tensor("Wd", [I, H], bf16, kind="ExternalInput").ap()
    yT = nc.dram_tensor("yT", [H, SW], bf16, kind="ExternalOutput").ap()
    dbg = nc.dram_tensor("dbg", [PB, 512], fp32, kind="ExternalOutput").ap()

    slot_segs = [_segs(w) for w in widths]
    slot_scs = [[s[i:i + 2] for i in range(0, len(s), 2)] for s in slot_segs]

    with tile.TileContext(nc) as tc:
        with tc.tile_pool(name="wp", bufs=1) as wp, \
             tc.tile_pool(name="xp", bufs=1) as xp, \
             tc.tile_pool(name="hp", bufs=1) as hp, \
             tc.tile_pool(name="evp", bufs=4) as evp, \
             tc.tile_pool(name="yp", bufs=4) as yp, \
             tc.tile_pool(name="psA", bufs=1, space="PSUM") as psA, \
             tc.tile_pool(name="psB", bufs=2, space="PSUM") as psB:

            # --- PE warmup / gap fillers: one long accumulation group of
            # dummy matmuls on a zeroed tile, batches interleaved with the
            # DMA-gated head so the HAM clock gate never sees an idle window.
            wz = wp.tile([PB, 512], bf16, name="wz")
            nc.gpsimd.memset(wz[:], 0)
            # fillers live in one psB "py0" slot (bufs=2): phase 2 of the
            # first superchunk starts after the filler group closes.
            pw = psB.tile([PB, 512], fp32, tag="py0", name="pw")
            fill_state = {"started": False, "closed": False}

            def filler(n, last=False):
                if fill_state["closed"]:
                    return
                for j in range(n):
                    nc.tensor.matmul(pw[:, 0:PB], wz[:, 0:PB], wz[:, 0:PB],
                                     start=(not fill_state["started"]),
                                     stop=last and (j == n - 1))
                    fill_state["started"] = True
                if last:
                    fill_state["closed"] = True
                    yw = wp.tile([PB, 512], fp32, name="yw")
                    nc.vector.tensor_copy(yw[:, 0:PB], pw[:, 0:PB])
                    nc.gpsimd.dma_start(out=dbg[:, 0:PB], in_=yw[:, 0:PB])

            filler(36)

            # --- DMA schedule.  Each dma_start costs the issuing sequencer
            # ~600ns regardless of size, so transfers are column-
            # consolidated: weights q1-3 as single [128,1536] rows, x as one
            # [128, w] tile per (k, superchunk).  x tiles use a depth-3 tag
            # pipeline (mod-3 tags, bufs=1): an x issue unblocks 3
            # superchunks ahead of need, so issue bursts at buffer-release
            # points never starve anything downstream in the queue.
            xsc = {}   # (k, global_sc) -> tile
            wg0, wu0, wgR, wuR = {}, {}, {}, {}
            wd_s = {}

            # global superchunk list: (slot, sci, col0, width)
            gsc = []
            for sj in range(NSLOT):
                for sci, sc in enumerate(slot_scs[sj]):
                    col0 = offs[sj] + sc[0][0]
                    wtot = sum(w for _, w in sc)
                    gsc.append((sj, sci, col0, wtot))

            def dma_x_sc(g):
                sj, sci, col0, wtot = gsc[g]
                for k in range(KB):
                    xt = xp.tile([PB, wtot], bf16, tag=f"x{k}_{g % 3}",
                                 name=f"x{k}_g{g}")
                    nc.sync.dma_start(out=xt[:],
                                      in_=xT[k * PB:(k + 1) * PB,
                                             col0:col0 + wtot])
                    xsc[(k, g)] = xt

            def dma_wd(q):
                for ib in range(q * QIB, (q + 1) * QIB):
                    wdt = wp.tile([PB, H], bf16, name=f"wd{ib}")
                    nc.sync.dma_start(out=wdt[:],
                                      in_=Wd[ib * PB:(ib + 1) * PB, :])
                    wd_s[ib] = wdt

            # Head: x (sc0, both segs in one tile per k) interleaved with
            # Wg q0, then Wu q0.
            _, _, col00, w00 = gsc[0]
            for k in range(KB):
                xt = xp.tile([PB, w00], bf16, tag=f"x{k}_0", name=f"x{k}_g0")
                nc.sync.dma_start(out=xt[:],
                                  in_=xT[k * PB:(k + 1) * PB,
                                         col00:col00 + w00])
                xsc[(k, 0)] = xt
                wgt = wp.tile([PB, QI], bf16, name=f"wg{k}_0")
                nc.sync.dma_start(out=wgt[:], in_=Wg[k * PB:(k + 1) * PB,
                                                     0:QI])
                wg0[k] = wgt
            for k in range(KB):
                wut = wp.tile([PB, QI], bf16, name=f"wu{k}_0")
                nc.sync.dma_start(out=wut[:], in_=Wu[k * PB:(k + 1) * PB,
                                                     0:QI])
                wu0[k] = wut
            dma_wd(0)
            dma_x_sc(1)
            for k in range(KB):
                wgt = wp.tile([PB, I - QI], bf16, name=f"wgR{k}")
                nc.sync.dma_start(out=wgt[:],
                                  in_=Wg[k * PB:(k + 1) * PB, QI:I])
                wgR[k] = wgt
            for k in range(KB):
                wut = wp.tile([PB, I - QI], bf16, name=f"wuR{k}")
                nc.sync.dma_start(out=wut[:],
                                  in_=Wu[k * PB:(k + 1) * PB, QI:I])
                wuR[k] = wut
            dma_x_sc(2)
            dma_wd(1)
            dma_wd(2)
            dma_wd(3)
            for g in range(3, len(gsc)):
                dma_x_sc(g)

            def wg_slice(k, q, lo, hi):
                if q == 0:
                    return wg0[k][:, lo:hi]
                return wgR[k][:, (q - 1) * QI + lo:(q - 1) * QI + hi]

            def wu_slice(k, q, lo, hi):
                if q == 0:
                    return wu0[k][:, lo:hi]
                return wuR[k][:, (q - 1) * QI + lo:(q - 1) * QI + hi]

            # --- fused compute, one slot (quarter-expert) at a time
            h_tiles = {}   # (g, ib, sl) -> tile
            sx_tiles = {}

            def emit_pg(g, ib, sl, sl_tag):
                sj, sci, col0, wtot = gsc[g]
                sc = slot_scs[sj][sci]
                t0, w = sc[sl]
                lo = t0 - sc[0][0]
                pg = psA.tile([PB, w], fp32, tag=f"pg{sl_tag % 2}",
                              name=f"pg{sl_tag}")
                for k in range(KB):
                    nc.tensor.matmul(
                        pg[:],
                        wg_slice(k, sj, ib * PB, (ib + 1) * PB),
                        xsc[(k, g)][:, lo:lo + w],
                        start=(k == 0), stop=(k == KB - 1))
                sx = evp.tile([PB, w], fp32, tag=f"sx{sl_tag % 2}",
                              name=f"sx{sl_tag}")
                nc.scalar.activation(sx[:], pg[:], AF.Silu)
                sx_tiles[(g, ib, sl)] = sx

            def emit_pu(g, ib, sl, sl_tag):
                sj, sci, col0, wtot = gsc[g]
                sc = slot_scs[sj][sci]
                t0, w = sc[sl]
                lo = t0 - sc[0][0]
                pu = psA.tile([PB, w], fp32, tag=f"pu{sl_tag % 2}",
                              name=f"pu{sl_tag}")
                for k in range(KB):
                    nc.tensor.matmul(
                        pu[:],
                        wu_slice(k, sj, ib * PB, (ib + 1) * PB),
                        xsc[(k, g)][:, lo:lo + w],
                        start=(k == 0), stop=(k == KB - 1))
                hh = hp.tile([PB, w], bf16, tag=f"h{ib}_{sl_tag}",
                             name=f"h{ib}_{sl_tag}")
                nc.vector.tensor_mul(hh[:], sx_tiles.pop((g, ib, sl))[:],
                                     pu[:])
                h_tiles[(g, ib, sl)] = hh

            def emit_p1(g, tiny=False):
                sj, sci, _, _ = gsc[g]
                sc = slot_scs[sj][sci]
                if g == 0:
                    # Head: pg-groups first (need only x g0 + Wg q0 =
                    # first 3 MB of DMA); filler batches bridge the
                    # DMA-gated stretches so HAM stays warm.
                    for sl in range(len(sc)):
                        for ib in range(QIB):
                            emit_pg(g, ib, sl, sl)
                        filler(16)
                        for ib in range(QIB):
                            emit_pu(g, ib, sl, sl)
                        if sl == 0:
                            filler(16)
                    filler(8, last=True)
                else:
                    for ib in range(QIB):
                        for sl in range(len(sc)):
                            sl_tag = 2 if tiny else sl
                            emit_pg(g, ib, sl, sl_tag)
                            emit_pu(g, ib, sl, sl_tag)

            def emit_p2(g):
                # Phase 2: y = h @ Wd (gates applied on host).  Both
                # segments' evictions share one yt tile and one DMA.
                sj, sci, col0, wtot = gsc[g]
                sc = slot_scs[sj][sci]
                for hb in range(HB):
                    yt = yp.tile([PB, wtot], bf16, tag="yt", name="yt")
                    for sl, (t0, w) in enumerate(sc):
                        lo = t0 - sc[0][0]
                        py = psB.tile([PB, w], fp32, tag=f"py{sl}",
                                      name=f"py{sl}")
                        for il in range(QIB):
                            nc.tensor.matmul(
                                py[:],
                                wd_s[sj * QIB + il][:,
                                                    hb * PB:(hb + 1) * PB],
                                h_tiles[(g, il, sl)][:],
                                start=(il == 0), stop=(il == QIB - 1))
                        nc.vector.tensor_copy(yt[:, lo:lo + w], py[:])
                    # Last two superchunks' y-stores flush on the sync HWDGE
                    # queue (idle once x loads finish): the gpsimd SWDGE
                    # backlog otherwise keeps the final drain waiting ~8us.
                    eng = nc.sync if g >= len(gsc) - 2 else nc.gpsimd
                    eng.dma_start(
                        out=yT[hb * PB:(hb + 1) * PB, col0:col0 + wtot],
                        in_=yt[:])

            # Plain superchunk order.  (Hoisting a trailing single-seg
            # superchunk's phase 1 ahead of the previous phase 2 was tried
            # to hide its silu/h chain, but perturbed the scheduler and
            # measured ~2-3us worse overall.)
            for g in range(len(gsc)):
                emit_p1(g)
                emit_p2(g)

    nc.compile()
    return nc


def _route(x, Wr, br):
    """Replicate the reference's fp32 router bit-compatibly on host."""
    logits = x @ Wr + br                       # fp32 GEMM
    order = np.argsort(-logits, axis=1, kind="stable")  # ties -> lowest index
    topk_idx = order[:, :TOPK]
    topk_vals = np.take_along_axis(logits, topk_idx, axis=1)
    g = 1.0 / (1.0 + np.exp(-topk_vals.astype(np.float32)))
    g = g / (np.sum(g, axis=-1, keepdims=True) + 1e-10)
    return topk_idx, g.astype(np.float32)


def kernel(x, Wr, br, Wg, Wu, Wd):
    global last_results
    from concourse.bass_utils import run_bass_kernel_spmd

    x = np.asarray(x, dtype=np.float32)
    Wr = np.asarray(Wr, dtype=np.float32)
    br = np.asarray(br, dtype=np.float32)
    Wg = np.asarray(Wg, dtype=np.float32)
    Wu = np.asarray(Wu, dtype=np.float32)
    Wd = np.asarray(Wd, dtype=np.float32)

    topk_idx, g = _route(x, Wr, br)

    idx_lists = []
    gate_lists = []
    for e in range(E):
        mask = topk_idx == e                    # [T, K]
        tok = np.nonzero(mask.any(axis=1))[0]
        gsel = np.where(mask[tok, 0], g[tok, 0], g[tok, 1]).astype(np.float32)
        idx_lists.append(tok.astype(np.int64))
        gate_lists.append(gsel)

    counts = np.array([len(ix) for ix in idx_lists])
    ranked = np.argsort(-counts, kind="stable")
    # slot j: experts ranked[2j] (cores 0-3) / ranked[2j+1] (cores 4-7);
    # core c hosts quarter (c % 4) of each of its slot experts.
    widths = tuple(max(512, int(counts[ranked[2 * j]])) for j in range(NSLOT))
    offs = [sum(widths[:j]) for j in range(NSLOT)]
    SW = sum(widths)

    key = widths
    if key not in _compiled:
        _compiled[key] = _build(widths)
    nc = _compiled[key]

    xTb = np.ascontiguousarray(x.T).astype(BF16)   # [H, T] bf16
    Wg16 = [Wg[e].astype(BF16) for e in range(E)]
    Wu16 = [Wu[e].astype(BF16) for e in range(E)]
    Wd16 = [Wd[e].astype(BF16) for e in range(E)]

    in_maps = []
    slot_expert = np.zeros((NCORES, NSLOT), dtype=int)
    for c in range(NCORES):
        qt = c % 4
        xTe = np.zeros((H, SW), dtype=BF16)
        Wg_in = np.zeros((H, I), dtype=BF16)
        Wu_in = np.zeros((H, I), dtype=BF16)
        Wd_in = np.zeros((I, H), dtype=BF16)
        for j in range(NSLOT):
            e = int(ranked[2 * j + (0 if c < 4 else 1)])
            slot_expert[c, j] = e
            n = counts[e]
            xTe[:, offs[j]:offs[j] + n] = xTb[:, idx_lists[e]]
            Wg_in[:, j * QI:(j + 1) * QI] = Wg16[e][:, qt * QI:(qt + 1) * QI]
            Wu_in[:, j * QI:(j + 1) * QI] = Wu16[e][:, qt * QI:(qt + 1) * QI]
            Wd_in[j * QI:(j + 1) * QI, :] = Wd16[e][qt * QI:(qt + 1) * QI, :]
        in_maps.append({"xT": xTe, "Wg": Wg_in, "Wu": Wu_in, "Wd": Wd_in})

    trace = bool(int(os.environ.get("MOE_TRACE", "0")))
    trace_cores = (list(range(NCORES))
                   if os.environ.get("MOE_TRACE_ALL") else None)
    last_results = run_bass_kernel_spmd(
        nc, in_maps, core_ids=list(range(NCORES)), trace=trace,
        trace_cores=trace_cores)

    out = np.zeros((T, H), dtype=np.float32)
    for j in range(NSLOT):
        for half, cores in ((0, range(0, 4)), (1, range(4, 8))):
            e = int(ranked[2 * j + half])
            n = counts[e]
            acc = np.zeros((n, H), dtype=np.float32)
            for c in cores:
                yTe = last_results.results[c]["yT"]
                acc += yTe[:, offs[j]:offs[j] + n].T.astype(np.float32)
            out[idx_lists[e]] += acc * gate_lists[e][:, None]
    return out


# revision 41
# speedup vs baseline: 1.0304x; 1.0009x over previous
"""MoE (top-2 of 8 experts, SwiGLU) Trainium2 kernel.

Strategy (quarter-sliced expert parallelism over 8 NeuronCores):
  * SwiGLU is elementwise in the intermediate dim I, so an expert's FFN
    splits cleanly into 4 independent I-slices of 512: each slice computes
    h_q = silu(x@Wg[:,q]) * (x@Wu[:,q]); y = sum_q h_q @ Wd[q,:].  Each core
    hosts 4 quarter-experts (12 MB bf16, same as one full expert).  Experts
    are paired by sorted token count ((1st,2nd), (3rd,4th), ...) and each
    pair's slices spread over one 4-core group, so per-core token capacity
    drops from max_e count_e (2182) to c1+c3+c5+c7 quarters (8278/4 ~ 2070
    token-equivalents) - near-perfect load balance.
  * Host: router GEMM + top-2 + sigmoid gates in numpy (matches the jax fp32
    reference bit-compatibly); gathers per-expert tokens into slot-packed
    bf16 buffers; applies gates and sums the 4 quarter partials on the host
    during the final scatter-add (all fp32).
  * Device (SPMD, per core): fused single pass per slot, all matmuls bf16
    (full PE rate; bf16 self-loading matmuls have no per-instruction weight
    penalty, unlike fp32r's ~13.5ns).  Weights stay SBUF-resident; x streams
    per superchunk; h lives in SBUF (no DRAM spill); y written as bf16.
    Dummy N=128 matmuls on a zeroed tile fill the unavoidable ~10us DMA head
    so the PE's HAM clock gate stays warm.
"""

import os
import numpy as np
import ml_dtypes

T, H, I, E, TOPK = 8192, 1024, 2048, 8, 2
NCORES = 8
PB = 128
NSLOT = 4
QI = I // NSLOT          # 512: I-columns per slot
QIB = QI // PB           # 4 i-blocks per slot

_compiled = {}
last_results = None  # BassKernelResults of the most recent run (for test harness)

BF16 = ml_dtypes.bfloat16


def _segs(width):
    segs = []
    t0 = 0
    while t0 < width:
        w = min(512, width - t0)
        segs.append((t0, w))
        t0 += w
    return segs


def _build(widths):
    import concourse.bacc as bacc
    import concourse.mybir as mybir
    import concourse.tile as tile

    fp32 = mybir.dt.float32
    bf16 = mybir.dt.bfloat16
    AF = mybir.ActivationFunctionType

    KB = H // PB   # 8 contraction blocks over H
    HB = H // PB   # 8 output blocks over H
    SW = sum(widths)
    offs = [sum(widths[:j]) for j in range(NSLOT)]

    nc = bacc.Bacc("TRN2", target_bir_lowering=False, debug=False,
                   num_devices=NCORES)
    xT = nc.dram_tensor("xT", [H, SW], bf16, kind="ExternalInput").ap()
    Wg = nc.dram_tensor("Wg", [H, I], bf16, kind="ExternalInput").ap()
    Wu = nc.dram_tensor("Wu", [H, I], bf16, kind="ExternalInput").ap()
    Wd = nc.dram_tensor("Wd", [I, H], bf16, kind="ExternalInput").ap()
    yT = nc.dram_tensor("yT", [H, SW], bf16, kind="ExternalOutput").ap()
    dbg = nc.dram_tensor("dbg", [PB, 512], fp32, kind="ExternalOutput").ap()

    slot_segs = [_segs(w) for w in widths]
    slot_scs = [[s[i:i + 2] for i in range(0, len(s), 2)] for s in slot_segs]

    with tile.TileContext(nc) as tc:
        with tc.tile_pool(name="wp", bufs=1) as wp, \
             tc.tile_pool(name="xp", bufs=1) as xp, \
             tc.tile_pool(name="hp", bufs=1) as hp, \
             tc.tile_pool(name="evp", bufs=4) as evp, \
             tc.tile_pool(name="yp", bufs=4) as yp, \
             tc.tile_pool(name="psA", bufs=1, space="PSUM") as psA, \
             tc.tile_pool(name="psB", bufs=2, space="PSUM") as psB:

            # --- PE warmup / gap fillers: one long accumulation group of
            # dummy matmuls on a zeroed tile, batches interleaved with the
            # DMA-gated head so the HAM clock gate never sees an idle window.
            wz = wp.tile([PB, 512], bf16, name="wz")
            nc.gpsimd.memset(wz[:], 0)
            # fillers live in one psB "py0" slot (bufs=2): phase 2 of the
            # first superchunk starts after the filler group closes.
            pw = psB.tile([PB, 512], fp32, tag="py0", name="pw")
            fill_state = {"started": False, "closed": False}

            def filler(n, last=False):
                if fill_state["closed"]:
                    return
                for j in range(n):
                    nc.tensor.matmul(pw[:, 0:PB], wz[:, 0:PB], wz[:, 0:PB],
                                     start=(not fill_state["started"]),
                                     stop=last and (j == n - 1))
                    fill_state["started"] = True
                if last:
                    fill_state["closed"] = True
                    yw = wp.tile([PB, 512], fp32, name="yw")
                    nc.vector.tensor_copy(yw[:, 0:PB], pw[:, 0:PB])
                    nc.gpsimd.dma_start(out=dbg[:, 0:PB], in_=yw[:, 0:PB])

            filler(36)

            # --- DMA schedule.  Each dma_start costs the issuing sequencer
            # ~600ns regardless of size, so transfers are column-
            # consolidated: weights q1-3 as single [128,1536] rows, x as one
            # [128, w] tile per (k, superchunk).  x tiles use a depth-3 tag
            # pipeline (mod-3 tags, bufs=1): an x issue unblocks 3
            # superchunks ahead of need, so issue bursts at buffer-release
            # points never starve anything downstream in the queue.
            xsc = {}   # (k, global_sc) -> tile
            wg0, wu0, wgR, wuR = {}, {}, {}, {}
            wd_s = {}

            # global superchunk list: (slot, sci, col0, width)
            gsc = []
            for sj in range(NSLOT):
                for sci, sc in enumerate(slot_scs[sj]):
                    col0 = offs[sj] + sc[0][0]
                    wtot = sum(w for _, w in sc)
                    gsc.append((sj, sci, col0, wtot))

            def dma_x_sc(g):
                sj, sci, col0, wtot = gsc[g]
                for k in range(KB):
                    xt = xp.tile([PB, wtot], bf16, tag=f"x{k}_{g % 3}",
                                 name=f"x{k}_g{g}")
                    nc.sync.dma_start(out=xt[:],
                                      in_=xT[k * PB:(k + 1) * PB,
                                             col0:col0 + wtot])
                    xsc[(k, g)] = xt

            def dma_wd(q):
                for ib in range(q * QIB, (q + 1) * QIB):
                    wdt = wp.tile([PB, H], bf16, name=f"wd{ib}")
                    nc.sync.dma_start(out=wdt[:],
                                      in_=Wd[ib * PB:(ib + 1) * PB, :])
                    wd_s[ib] = wdt

            # Head: x (sc0, both segs in one tile per k) interleaved with
            # Wg q0, then Wu q0.
            _, _, col00, w00 = gsc[0]
            for k in range(KB):
                xt = xp.tile([PB, w00], bf16, tag=f"x{k}_0", name=f"x{k}_g0")
                nc.sync.dma_start(out=xt[:],
                                  in_=xT[k * PB:(k + 1) * PB,
                                         col00:col00 + w00])
                xsc[(k, 0)] = xt
                wgt = wp.tile([PB, QI], bf16, name=f"wg{k}_0")
                nc.sync.dma_start(out=wgt[:], in_=Wg[k * PB:(k + 1) * PB,
                                                     0:QI])
                wg0[k] = wgt
            for k in range(KB):
                wut = wp.tile([PB, QI], bf16, name=f"wu{k}_0")
                nc.sync.dma_start(out=wut[:], in_=Wu[k * PB:(k + 1) * PB,
                                                     0:QI])
                wu0[k] = wut
            dma_wd(0)
            dma_x_sc(1)
            for k in range(KB):
                wgt = wp.tile([PB, I - QI], bf16, name=f"wgR{k}")
                nc.sync.dma_start(out=wgt[:],
                                  in_=Wg[k * PB:(k + 1) * PB, QI:I])
                wgR[k] = wgt
            for k in range(KB):
                wut = wp.tile([PB, I - QI], bf16, name=f"wuR{k}")
                nc.sync.dma_start(out=wut[:],
                                  in_=Wu[k * PB:(k + 1) * PB, QI:I])
                wuR[k] = wut
            dma_x_sc(2)
            dma_wd(1)
            dma_wd(2)
            dma_wd(3)
            for g in range(3, len(gsc)):
                dma_x_sc(g)

            def wg_slice(k, q, lo, hi):
                if q == 0:
                    return wg0[k][:, lo:hi]
                return wgR[k][:, (q - 1) * QI + lo:(q - 1) * QI + hi]

            def wu_slice(k, q, lo, hi):
                if q == 0:
                    return wu0[k][:, lo:hi]
                return wuR[k][:, (q - 1) * QI + lo:(q - 1) * QI + hi]

            # --- fused compute, one slot (quarter-expert) at a time
            h_tiles = {}   # (g, ib, sl) -> tile
            sx_tiles = {}

            def emit_pg(g, ib, sl, sl_tag):
                sj, sci, col0, wtot = gsc[g]
                sc = slot_scs[sj][sci]
                t0, w = sc[sl]
                lo = t0 - sc[0][0]
                ti = (ib % 2) if len(sc) == 1 else (sl_tag % 2)
                pg = psA.tile([PB, w], fp32, tag=f"pg{ti}",
                              name=f"pg{sl_tag}")
                for k in range(KB):
                    nc.tensor.matmul(
                        pg[:],
                        wg_slice(k, sj, ib * PB, (ib + 1) * PB),
                        xsc[(k, g)][:, lo:lo + w],
                        start=(k == 0), stop=(k == KB - 1))
                sx = evp.tile([PB, w], fp32, tag=f"sx{ti}",
                              name=f"sx{sl_tag}")
                nc.scalar.activation(sx[:], pg[:], AF.Silu)
                sx_tiles[(g, ib, sl)] = sx

            def emit_pu(g, ib, sl, sl_tag):
                sj, sci, col0, wtot = gsc[g]
                sc = slot_scs[sj][sci]
                t0, w = sc[sl]
                lo = t0 - sc[0][0]
                ti = (ib % 2) if len(sc) == 1 else (sl_tag % 2)
                pu = psA.tile([PB, w], fp32, tag=f"pu{ti}",
                              name=f"pu{sl_tag}")
                for k in range(KB):
                    nc.tensor.matmul(
                        pu[:],
                        wu_slice(k, sj, ib * PB, (ib + 1) * PB),
                        xsc[(k, g)][:, lo:lo + w],
                        start=(k == 0), stop=(k == KB - 1))
                hh = hp.tile([PB, w], bf16, tag=f"h{ib}_{sl_tag}",
                             name=f"h{ib}_{sl_tag}")
                nc.vector.tensor_mul(hh[:], sx_tiles.pop((g, ib, sl))[:],
                                     pu[:])
                h_tiles[(g, ib, sl)] = hh

            def emit_p1(g, tiny=False):
                sj, sci, _, _ = gsc[g]
                sc = slot_scs[sj][sci]
                if g == 0:
                    # Head: pg-groups first (need only x g0 + Wg q0 =
                    # first 3 MB of DMA); filler batches bridge the
                    # DMA-gated stretches so HAM stays warm.
                    for sl in range(len(sc)):
                        for ib in range(QIB):
                            emit_pg(g, ib, sl, sl)
                        filler(16)
                        for ib in range(QIB):
                            emit_pu(g, ib, sl, sl)
                        if sl == 0:
                            filler(16)
                    filler(8, last=True)
                else:
                    for ib in range(QIB):
                        for sl in range(len(sc)):
                            sl_tag = 2 if tiny else sl
                            emit_pg(g, ib, sl, sl_tag)
                            emit_pu(g, ib, sl, sl_tag)

            def emit_p2(g):
                # Phase 2: y = h @ Wd (gates applied on host).  Both
                # segments' evictions share one yt tile and one DMA.
                sj, sci, col0, wtot = gsc[g]
                sc = slot_scs[sj][sci]
                for hb in range(HB):
                    yt = yp.tile([PB, wtot], bf16, tag="yt", name="yt")
                    for sl, (t0, w) in enumerate(sc):
                        lo = t0 - sc[0][0]
                        pyi = (hb % 2) if len(sc) == 1 else sl
                        py = psB.tile([PB, w], fp32, tag=f"py{pyi}",
                                      name=f"py{sl}")
                        for il in range(QIB):
                            nc.tensor.matmul(
                                py[:],
                                wd_s[sj * QIB + il][:,
                                                    hb * PB:(hb + 1) * PB],
                                h_tiles[(g, il, sl)][:],
                                start=(il == 0), stop=(il == QIB - 1))
                        nc.vector.tensor_copy(yt[:, lo:lo + w], py[:])
                    # Last two superchunks' y-stores flush on the sync HWDGE
                    # queue (idle once x loads finish): the gpsimd SWDGE
                    # backlog otherwise keeps the final drain waiting ~8us.
                    eng = nc.sync if g >= len(gsc) - 2 else nc.gpsimd
                    eng.dma_start(
                        out=yT[hb * PB:(hb + 1) * PB, col0:col0 + wtot],
                        in_=yt[:])

            # Plain superchunk order.  (Hoisting a trailing single-seg
            # superchunk's phase 1 ahead of the previous phase 2 was tried
            # to hide its silu/h chain, but perturbed the scheduler and
            # measured ~2-3us worse overall.)
            for g in range(len(gsc)):
                emit_p1(g)
                emit_p2(g)

    nc.compile()
    return nc


def _route(x, Wr, br):
    """Replicate the reference's fp32 router bit-compatibly on host."""
    logits = x @ Wr + br                       # fp32 GEMM
    order = np.argsort(-logits, axis=1, kind="stable")  # ties -> lowest index
    topk_idx = order[:, :TOPK]
    topk_vals = np.take_along_axis(logits, topk_idx, axis=1)
    g = 1.0 / (1.0 + np.exp(-topk_vals.astype(np.float32)))
    g = g / (np.sum(g, axis=-1, keepdims=True) + 1e-10)
    return topk_idx, g.astype(np.float32)


def kernel(x, Wr, br, Wg, Wu, Wd):
    global last_results
    from concourse.bass_utils import run_bass_kernel_spmd

    x = np.asarray(x, dtype=np.float32)
    Wr = np.asarray(Wr, dtype=np.float32)
    br = np.asarray(br, dtype=np.float32)
    Wg = np.asarray(Wg, dtype=np.float32)
    Wu = np.asarray(Wu, dtype=np.float32)
    Wd = np.asarray(Wd, dtype=np.float32)

    topk_idx, g = _route(x, Wr, br)

    idx_lists = []
    gate_lists = []
    for e in range(E):
        mask = topk_idx == e                    # [T, K]
        tok = np.nonzero(mask.any(axis=1))[0]
        gsel = np.where(mask[tok, 0], g[tok, 0], g[tok, 1]).astype(np.float32)
        idx_lists.append(tok.astype(np.int64))
        gate_lists.append(gsel)

    counts = np.array([len(ix) for ix in idx_lists])
    ranked = np.argsort(-counts, kind="stable")
    # slot j: experts ranked[2j] (cores 0-3) / ranked[2j+1] (cores 4-7);
    # core c hosts quarter (c % 4) of each of its slot experts.
    widths = tuple(max(512, int(counts[ranked[2 * j]])) for j in range(NSLOT))
    offs = [sum(widths[:j]) for j in range(NSLOT)]
    SW = sum(widths)

    key = widths
    if key not in _compiled:
        _compiled[key] = _build(widths)
    nc = _compiled[key]

    xTb = np.ascontiguousarray(x.T).astype(BF16)   # [H, T] bf16
    Wg16 = [Wg[e].astype(BF16) for e in range(E)]
    Wu16 = [Wu[e].astype(BF16) for e in range(E)]
    Wd16 = [Wd[e].astype(BF16) for e in range(E)]

    in_maps = []
    slot_expert = np.zeros((NCORES, NSLOT), dtype=int)
    for c in range(NCORES):
        qt = c % 4
        xTe = np.zeros((H, SW), dtype=BF16)
        Wg_in = np.zeros((H, I), dtype=BF16)
        Wu_in = np.zeros((H, I), dtype=BF16)
        Wd_in = np.zeros((I, H), dtype=BF16)
        for j in range(NSLOT):
            e = int(ranked[2 * j + (0 if c < 4 else 1)])
            slot_expert[c, j] = e
            n = counts[e]
            xTe[:, offs[j]:offs[j] + n] = xTb[:, idx_lists[e]]
            Wg_in[:, j * QI:(j + 1) * QI] = Wg16[e][:, qt * QI:(qt + 1) * QI]
            Wu_in[:, j * QI:(j + 1) * QI] = Wu16[e][:, qt * QI:(qt + 1) * QI]
            Wd_in[j * QI:(j + 1) * QI, :] = Wd16[e][qt * QI:(qt + 1) * QI, :]
        in_maps.append({"xT": xTe, "Wg": Wg_in, "Wu": Wu_in, "Wd": Wd_in})

    trace = bool(int(os.environ.get("MOE_TRACE", "0")))
    trace_cores = (list(range(NCORES))
                   if os.environ.get("MOE_TRACE_ALL") else None)
    last_results = run_bass_kernel_spmd(
        nc, in_maps, core_ids=list(range(NCORES)), trace=trace,
        trace_cores=trace_cores)

    out = np.zeros((T, H), dtype=np.float32)
    for j in range(NSLOT):
        for half, cores in ((0, range(0, 4)), (1, range(4, 8))):
            e = int(ranked[2 * j + half])
            n = counts[e]
            acc = np.zeros((n, H), dtype=np.float32)
            for c in cores:
                yTe = last_results.results[c]["yT"]
                acc += yTe[:, offs[j]:offs[j] + n].T.astype(np.float32)
            out[idx_lists[e]] += acc * gate_lists[e][:, None]
    return out
